# revision 16
# baseline (speedup 1.0000x reference)
"""MoE (B=2,T=2048,D=768,E=8,K=2,H=1536) Trainium2 kernel.

Sparse expert-parallel over the 8 NeuronCores: the host computes the gate
(softmax + top-2) in numpy, gathers the tokens routed to each expert, and
core e runs expert e's FFN only on its ~B*T*K/E gathered tokens. The
per-token gate weight is applied on device; the host scatter-adds the two
weighted expert outputs per token.

Activations stay feature-major (x^T [D, tok]) so gate/up banks [D,H] and
the down bank [H,D] are already in the stationary-operand (lhsT) layout the
PE wants — no transposes on device. The big GEMMs run in float32r (the PE's
single-pass fp32 mode, ~3.4x the 4-pass fp32 rate; per-GEMM rel err ~1.5e-4).
"""

import numpy as np

import concourse.bass as bass
import concourse.mybir as mybir
import concourse.tile as tile
from concourse import bass_utils

# Problem shape (hardcoded per contract).
B, T, D, E, H, KTOP = 2, 2048, 768, 8, 1536, 2
NTOK = B * T            # 4096 tokens
TOK = 512               # max tokens per block
DC = D // 128           # 6 chunks of the D (contraction) dim
HC = H // 128           # 12 chunks of the H dim
F32 = mybir.dt.float32
F32R = mybir.dt.float32r


def _install_axon_ntff_hook():
    """Best-effort: register the antenv.axon_hooks NTFF profile hook that the
    agent image lacks, so trace=True (or BASS_TRACE=1) can profile under axon.
    Never raises."""
    try:
        import sys, types, contextlib, ctypes  # noqa: PLC0415
        import antenv  # noqa: PLC0415
        if "antenv.axon_hooks" in sys.modules:
            return
        _HOOK = [None]
        mod = types.ModuleType("antenv.axon_hooks")
        mod.set_axon_ntff_profile_hook = lambda h: _HOOK.__setitem__(0, h)
        mod.get_axon_ntff_profile_hook = lambda: _HOOK[0]
        sys.modules["antenv.axon_hooks"] = mod
        antenv.axon_hooks = mod

        lib = ctypes.CDLL("/opt/axon/libaxon_pjrt.so")
        if not hasattr(lib, "axon_start_nrt_profile"):
            return
        lib.axon_start_nrt_profile.argtypes = [
            ctypes.POINTER(ctypes.c_int64), ctypes.c_size_t]
        lib.axon_start_nrt_profile.restype = ctypes.c_int64
        lib.axon_stop_nrt_profile.argtypes = [ctypes.c_char_p]
        lib.axon_stop_nrt_profile.restype = ctypes.c_int64

        @contextlib.contextmanager
        def _hook(output_dir, device_ids):
            import jax  # noqa: PLC0415
            jax.devices()
            if device_ids:
                ids = (ctypes.c_int64 * len(device_ids))(*device_ids)
                rc = lib.axon_start_nrt_profile(ids, len(device_ids))
            else:
                rc = lib.axon_start_nrt_profile(None, 0)
            if rc != 0:
                raise RuntimeError(f"axon_start_nrt_profile rc={rc}")
            try:
                yield
            finally:
                lib.axon_stop_nrt_profile(str(output_dir).encode())

        mod.set_axon_ntff_profile_hook(_hook)
    except Exception:
        pass


def _split_multiwaits(nc):
    """This walrus build only supports one sync-wait per instruction; move
    extra waits onto preceding NOPs on the same engine."""
    for fn in nc.m.functions:
        for bb in fn.blocks:
            out = []
            for ins in bb.instructions:
                si = ins.sync_info
                if si is not None and si.on_wait is not None and len(si.on_wait) > 1:
                    waits = list(si.on_wait)
                    for i, w in enumerate(waits[:-1]):
                        out.append(mybir.InstNoOp(
                            name=f"{ins.name}-sw{i}",
                            engine=ins.engine,
                            sync_info=mybir.SyncInfo(on_wait=[w], on_update=[]),
                        ))
                    si.on_wait = [waits[-1]]
                    ins.sync_info = si
                out.append(ins)
            bb.instructions = out
    return nc


def build_nc(npad):
    """Expert FFN on `npad` gathered tokens (feature-major, f32r GEMMs)."""
    # Equal-ish blocks of at most TOK tokens (multiples of 128): balanced
    # blocks beat [512, 512, remainder] because per-block matmul count is
    # fixed while per-matmul cost scales with N.
    ntile = npad // 128
    nblk = -(-ntile // (TOK // 128))
    sizes = [(ntile // nblk + (1 if i < ntile % nblk else 0)) * 128
             for i in range(nblk)]
    blocks = []
    off = 0
    for s in sizes:
        blocks.append((off, s))
        off += s

    nc = bass.Bass()
    xgT = nc.dram_tensor("xgT", [D, npad], F32R, kind="ExternalInput")
    gb = nc.dram_tensor("gb", [D, H], F32R, kind="ExternalInput")
    ub = nc.dram_tensor("ub", [D, H], F32R, kind="ExternalInput")
    db = nc.dram_tensor("db", [H, D], F32R, kind="ExternalInput")
    wrow = nc.dram_tensor("wrow", [1, npad], F32R, kind="ExternalInput")
    onesd = nc.dram_tensor("onesd", [1, 128], F32R, kind="ExternalInput")
    ygT = nc.dram_tensor("ygT", [D, npad], F32, kind="ExternalOutput")

    xgT_r = xgT.rearrange("(c p) t -> p c t", p=128)   # [128, DC, npad]
    gb_r = gb.rearrange("(c p) h -> p c h", p=128)     # [128, DC, H]
    ub_r = ub.rearrange("(c p) h -> p c h", p=128)
    db_r = db.rearrange("(c p) d -> p c d", p=128)     # [128, HC, D]
    ygT_r = ygT.rearrange("(c p) t -> p c t", p=128)

    with tile.TileContext(nc) as tc:
        with (
            tc.tile_pool(name="wts", bufs=1) as wts,
            tc.tile_pool(name="xp", bufs=2) as xp,
            tc.tile_pool(name="hp", bufs=18) as hp,
            tc.tile_pool(name="sap", bufs=2) as sap,
            tc.tile_pool(name="yp", bufs=3) as yp,
            tc.tile_pool(name="wsp", bufs=2) as wsp,
            tc.tile_pool(name="ps", bufs=8, space="PSUM") as ps,
        ):
            # Resident expert banks (f32r straight from DRAM), chunked and on
            # the ACT HWDGE ring (nc.scalar) so the x-block / output DMAs on
            # the SP ring (nc.sync) don't queue behind 14 MB of weights.
            # gb/ub interleave per D-chunk so GEMM1's k-loop streams as the
            # chunks land; db follows (needed only when GEMM2 starts).
            wrow_sb = wts.tile([1, npad], F32R)
            nc.scalar.dma_start(wrow_sb[:], wrow[:])
            gb_k = [wts.tile([128, H], F32R, tag=f"gb{k}", name=f"gb{k}")
                    for k in range(DC)]
            ub_k = [wts.tile([128, H], F32R, tag=f"ub{k}", name=f"ub{k}")
                    for k in range(DC)]
            for k in range(DC):
                nc.scalar.dma_start(gb_k[k][:], gb_r[:, k, :])
                nc.scalar.dma_start(ub_k[k][:], ub_r[:, k, :])
            db_k = [wts.tile([128, D], F32R, tag=f"db{k}", name=f"db{k}")
                    for k in range(HC)]
            for k in range(HC):
                nc.scalar.dma_start(db_k[k][:], db_r[:, k, :])
            ones_sb = wts.tile([1, 128], F32R)
            nc.scalar.dma_start(ones_sb[:], onesd[:])

            for off, tb in blocks:
                blk = slice(off, off + tb)

                xb = xp.tile([128, DC, tb], F32R, tag="xb")
                nc.sync.dma_start(xb[:], xgT_r[:, :, blk])

                # Broadcast the per-token gate weight across 128 partitions:
                # W[p, t] = wrow[t] via ones[1,128].T @ wrow[1, tb].
                w_ps = ps.tile([128, tb], F32, tag="ps")
                nc.tensor.matmul(w_ps[:], ones_sb[:], wrow_sb[:, blk],
                                 start=True, stop=True)
                wsb = wsp.tile([128, tb], F32)
                nc.vector.tensor_copy(wsb[:], w_ps[:])

                # h = silu(x@gb) * (x@ub), feature-major [H, tb]
                hts = []
                for ht in range(HC):
                    hsl = slice(ht * 128, (ht + 1) * 128)
                    a_ps = ps.tile([128, tb], F32, tag="ps")
                    for k in range(DC):
                        nc.tensor.matmul(a_ps[:], gb_k[k][:, hsl],
                                         xb[:, k, :],
                                         start=(k == 0), stop=(k == DC - 1))
                    u_ps = ps.tile([128, tb], F32, tag="ps")
                    for k in range(DC):
                        nc.tensor.matmul(u_ps[:], ub_k[k][:, hsl],
                                         xb[:, k, :],
                                         start=(k == 0), stop=(k == DC - 1))
                    sa = sap.tile([128, tb], F32)
                    nc.scalar.activation(sa[:], a_ps[:],
                                         mybir.ActivationFunctionType.Silu)
                    hch = hp.tile([128, tb], F32R, tag="h")
                    nc.vector.tensor_mul(hch[:], sa[:],
                                         u_ps[:])
                    hts.append(hch)

                # y^T = db^T @ h, scaled by the gate weight
                for dt in range(DC):
                    dsl = slice(dt * 128, (dt + 1) * 128)
                    y_ps = ps.tile([128, tb], F32, tag="ps")
                    for hk in range(HC):
                        nc.tensor.matmul(y_ps[:], db_k[hk][:, dsl],
                                         hts[hk][:],
                                         start=(hk == 0), stop=(hk == HC - 1))
                    ysb = yp.tile([128, tb], F32)
                    nc.vector.tensor_mul(ysb[:], y_ps[:],
                                         wsb[:])
                    nc.sync.dma_start(ygT_r[:, dt, blk], ysb[:])

    return _split_multiwaits(nc)


_NC_CACHE = {}


def _routing(x2d, gate_w):
    """Replicates the reference gate: softmax over E, top-2, renormalize."""
    logits = x2d @ gate_w.T                                  # [NTOK, E] f32
    lmax = logits.max(-1, keepdims=True)
    p = np.exp(logits - lmax)
    p = p / p.sum(-1, keepdims=True)
    idx = np.argsort(-p, axis=-1, kind="stable")[:, :KTOP]   # [NTOK, 2]
    sel = np.take_along_axis(p, idx, -1)
    w = sel / (sel.sum(-1, keepdims=True) + 1e-8)            # [NTOK, 2]
    return idx, w.astype(np.float32)


def kernel(x, gate_w, gate_bank, up_bank, down_bank, _trace=False):
    _install_axon_ntff_hook()
    x = np.asarray(x, dtype=np.float32)
    gate_w = np.asarray(gate_w, dtype=np.float32)
    x2d = np.ascontiguousarray(x.reshape(NTOK, D))

    idx, w = _routing(x2d, gate_w)

    # Token lists per expert.
    tok_idx = []
    tok_w = []
    for e in range(E):
        hit = (idx == e)                        # [NTOK, 2]
        rows = np.nonzero(hit.any(-1))[0]
        tok_idx.append(rows)
        tok_w.append(w[rows, np.argmax(hit[rows], axis=-1)])
    nmax = max(len(r) for r in tok_idx)
    npad = ((nmax + 127) // 128) * 128

    key = npad
    if key not in _NC_CACHE:
        _NC_CACHE[key] = build_nc(npad)
    nc = _NC_CACHE[key]

    in_maps = []
    for e in range(E):
        rows = tok_idx[e]
        xg = np.zeros((npad, D), np.float32)
        xg[: len(rows)] = x2d[rows]
        wr = np.zeros((1, npad), np.float32)
        wr[0, : len(rows)] = tok_w[e]
        in_maps.append({
            "xgT": np.ascontiguousarray(xg.T),
            "gb": np.ascontiguousarray(gate_bank[e], dtype=np.float32),
            "ub": np.ascontiguousarray(up_bank[e], dtype=np.float32),
            "db": np.ascontiguousarray(down_bank[e], dtype=np.float32),
            "wrow": wr,
            "onesd": np.ones((1, 128), np.float32),
        })

    res = bass_utils.run_bass_kernel_spmd(
        nc, in_maps, core_ids=list(range(8)), trace=_trace)

    y = np.zeros((NTOK, D), np.float32)
    for e in range(E):
        rows = tok_idx[e]
        y[rows] += res.results[e]["ygT"][:, : len(rows)].T
    y = y.reshape(B, T, D)
    if _trace:
        return y, res
    return y


# revision 17
# speedup vs baseline: 1.0889x; 1.0889x over previous
"""MoE (B=2,T=2048,D=768,E=8,K=2,H=1536) Trainium2 kernel.

Sparse expert-parallel over the 8 NeuronCores: the host computes the gate
(softmax + top-2) in numpy, gathers the tokens routed to each expert, and
core e runs expert e's FFN only on its ~B*T*K/E gathered tokens. The
per-token gate weight is applied on device; the host scatter-adds the two
weighted expert outputs per token.

Activations stay feature-major (x^T [D, tok]) so gate/up banks [D,H] and
the down bank [H,D] are already in the stationary-operand (lhsT) layout the
PE wants — no transposes on device. The big GEMMs run in float32r (the PE's
single-pass fp32 mode, ~3.4x the 4-pass fp32 rate; per-GEMM rel err ~1.5e-4).
"""

import numpy as np

import concourse.bass as bass
import concourse.mybir as mybir
import concourse.tile as tile
from concourse import bass_utils

# Problem shape (hardcoded per contract).
B, T, D, E, H, KTOP = 2, 2048, 768, 8, 1536, 2
NTOK = B * T            # 4096 tokens
TOK = 512               # max tokens per block
DC = D // 128           # 6 chunks of the D (contraction) dim
HC = H // 128           # 12 chunks of the H dim
F32 = mybir.dt.float32
F32R = mybir.dt.float32r


def _install_axon_ntff_hook():
    """Best-effort: register the antenv.axon_hooks NTFF profile hook that the
    agent image lacks, so trace=True (or BASS_TRACE=1) can profile under axon.
    Never raises."""
    try:
        import sys, types, contextlib, ctypes  # noqa: PLC0415
        import antenv  # noqa: PLC0415
        if "antenv.axon_hooks" in sys.modules:
            return
        _HOOK = [None]
        mod = types.ModuleType("antenv.axon_hooks")
        mod.set_axon_ntff_profile_hook = lambda h: _HOOK.__setitem__(0, h)
        mod.get_axon_ntff_profile_hook = lambda: _HOOK[0]
        sys.modules["antenv.axon_hooks"] = mod
        antenv.axon_hooks = mod

        lib = ctypes.CDLL("/opt/axon/libaxon_pjrt.so")
        if not hasattr(lib, "axon_start_nrt_profile"):
            return
        lib.axon_start_nrt_profile.argtypes = [
            ctypes.POINTER(ctypes.c_int64), ctypes.c_size_t]
        lib.axon_start_nrt_profile.restype = ctypes.c_int64
        lib.axon_stop_nrt_profile.argtypes = [ctypes.c_char_p]
        lib.axon_stop_nrt_profile.restype = ctypes.c_int64

        @contextlib.contextmanager
        def _hook(output_dir, device_ids):
            import jax  # noqa: PLC0415
            jax.devices()
            if device_ids:
                ids = (ctypes.c_int64 * len(device_ids))(*device_ids)
                rc = lib.axon_start_nrt_profile(ids, len(device_ids))
            else:
                rc = lib.axon_start_nrt_profile(None, 0)
            if rc != 0:
                raise RuntimeError(f"axon_start_nrt_profile rc={rc}")
            try:
                yield
            finally:
                lib.axon_stop_nrt_profile(str(output_dir).encode())

        mod.set_axon_ntff_profile_hook(_hook)
    except Exception:
        pass


def _split_multiwaits(nc):
    """This walrus build only supports one sync-wait per instruction; move
    extra waits onto preceding NOPs on the same engine."""
    for fn in nc.m.functions:
        for bb in fn.blocks:
            out = []
            for ins in bb.instructions:
                si = ins.sync_info
                if si is not None and si.on_wait is not None and len(si.on_wait) > 1:
                    waits = list(si.on_wait)
                    for i, w in enumerate(waits[:-1]):
                        out.append(mybir.InstNoOp(
                            name=f"{ins.name}-sw{i}",
                            engine=ins.engine,
                            sync_info=mybir.SyncInfo(on_wait=[w], on_update=[]),
                        ))
                    si.on_wait = [waits[-1]]
                    ins.sync_info = si
                out.append(ins)
            bb.instructions = out
    return nc


def build_nc(npad):
    """Expert FFN on `npad` gathered tokens (feature-major, f32r GEMMs)."""
    # Equal-ish blocks of at most TOK tokens (multiples of 128): balanced
    # blocks beat [512, 512, remainder] because per-block matmul count is
    # fixed while per-matmul cost scales with N.
    ntile = npad // 128
    nblk = -(-ntile // (TOK // 128))
    sizes = [(ntile // nblk + (1 if i < ntile % nblk else 0)) * 128
             for i in range(nblk)]
    blocks = []
    off = 0
    for s in sizes:
        blocks.append((off, s))
        off += s

    nc = bass.Bass()
    xgT = nc.dram_tensor("xgT", [D, npad], F32R, kind="ExternalInput")
    gb = nc.dram_tensor("gb", [D, H], F32R, kind="ExternalInput")
    ub = nc.dram_tensor("ub", [D, H], F32R, kind="ExternalInput")
    db = nc.dram_tensor("db", [H, D], F32R, kind="ExternalInput")
    wrow = nc.dram_tensor("wrow", [1, npad], F32R, kind="ExternalInput")
    onesd = nc.dram_tensor("onesd", [1, 128], F32R, kind="ExternalInput")
    ygT = nc.dram_tensor("ygT", [D, npad], F32, kind="ExternalOutput")

    xgT_r = xgT.rearrange("(c p) t -> p c t", p=128)   # [128, DC, npad]
    gb_r = gb.rearrange("(c p) h -> p c h", p=128)     # [128, DC, H]
    ub_r = ub.rearrange("(c p) h -> p c h", p=128)
    db_r = db.rearrange("(c p) d -> p c d", p=128)     # [128, HC, D]
    ygT_r = ygT.rearrange("(c p) t -> p c t", p=128)

    with tile.TileContext(nc) as tc:
        with (
            tc.tile_pool(name="wts", bufs=1) as wts,
            tc.tile_pool(name="xp", bufs=2) as xp,
            tc.tile_pool(name="hp", bufs=18) as hp,
            tc.tile_pool(name="sap", bufs=2) as sap,
            tc.tile_pool(name="yp", bufs=3) as yp,
            tc.tile_pool(name="wsp", bufs=2) as wsp,
            tc.tile_pool(name="ps", bufs=8, space="PSUM") as ps,
        ):
            # Resident expert banks (f32r straight from DRAM), chunked and on
            # the ACT HWDGE ring (nc.scalar) so the x-block / output DMAs on
            # the SP ring (nc.sync) don't queue behind 14 MB of weights.
            # gb/ub interleave per D-chunk so GEMM1's k-loop streams as the
            # chunks land; db follows (needed only when GEMM2 starts).
            ones_sb = wts.tile([1, 128], F32R)
            nc.scalar.dma_start(ones_sb[:], onesd[:])
            wrow_sb = wts.tile([1, npad], F32R)
            nc.scalar.dma_start(wrow_sb[:], wrow[:])
            gb_k = [wts.tile([128, H], F32R, tag=f"gb{k}", name=f"gb{k}")
                    for k in range(DC)]
            ub_k = [wts.tile([128, H], F32R, tag=f"ub{k}", name=f"ub{k}")
                    for k in range(DC)]
            for k in range(DC):
                nc.scalar.dma_start(gb_k[k][:], gb_r[:, k, :])
                nc.scalar.dma_start(ub_k[k][:], ub_r[:, k, :])
            db_k = [wts.tile([128, D], F32R, tag=f"db{k}", name=f"db{k}")
                    for k in range(HC)]
            for k in range(HC):
                nc.scalar.dma_start(db_k[k][:], db_r[:, k, :])

            for off, tb in blocks:
                blk = slice(off, off + tb)

                xb = xp.tile([128, DC, tb], F32R, tag="xb")
                nc.sync.dma_start(xb[:], xgT_r[:, :, blk])

                # Broadcast the per-token gate weight across 128 partitions:
                # W[p, t] = wrow[t] via ones[1,128].T @ wrow[1, tb].
                w_ps = ps.tile([128, tb], F32, tag="ps")
                nc.tensor.matmul(w_ps[:], ones_sb[:], wrow_sb[:, blk],
                                 start=True, stop=True)
                wsb = wsp.tile([128, tb], F32)
                nc.vector.tensor_copy(wsb[:], w_ps[:])

                # h = silu(x@gb) * (x@ub), feature-major [H, tb]
                hts = []
                for ht in range(HC):
                    hsl = slice(ht * 128, (ht + 1) * 128)
                    a_ps = ps.tile([128, tb], F32, tag="ps")
                    for k in range(DC):
                        nc.tensor.matmul(a_ps[:], gb_k[k][:, hsl],
                                         xb[:, k, :],
                                         start=(k == 0), stop=(k == DC - 1))
                    u_ps = ps.tile([128, tb], F32, tag="ps")
                    for k in range(DC):
                        nc.tensor.matmul(u_ps[:], ub_k[k][:, hsl],
                                         xb[:, k, :],
                                         start=(k == 0), stop=(k == DC - 1))
                    sa = sap.tile([128, tb], F32)
                    nc.scalar.activation(sa[:], a_ps[:],
                                         mybir.ActivationFunctionType.Silu)
                    hch = hp.tile([128, tb], F32R, tag="h")
                    nc.vector.tensor_mul(hch[:], sa[:],
                                         u_ps[:])
                    hts.append(hch)

                # y^T = db^T @ h, scaled by the gate weight
                for dt in range(DC):
                    dsl = slice(dt * 128, (dt + 1) * 128)
                    y_ps = ps.tile([128, tb], F32, tag="ps")
                    for hk in range(HC):
                        nc.tensor.matmul(y_ps[:], db_k[hk][:, dsl],
                                         hts[hk][:],
                                         start=(hk == 0), stop=(hk == HC - 1))
                    ysb = yp.tile([128, tb], F32)
                    nc.vector.tensor_mul(ysb[:], y_ps[:],
                                         wsb[:])
                    nc.sync.dma_start(ygT_r[:, dt, blk], ysb[:])

    return _split_multiwaits(nc)


_NC_CACHE = {}


def _routing(x2d, gate_w):
    """Replicates the reference gate: softmax over E, top-2, renormalize."""
    logits = x2d @ gate_w.T                                  # [NTOK, E] f32
    lmax = logits.max(-1, keepdims=True)
    p = np.exp(logits - lmax)
    p = p / p.sum(-1, keepdims=True)
    idx = np.argsort(-p, axis=-1, kind="stable")[:, :KTOP]   # [NTOK, 2]
    sel = np.take_along_axis(p, idx, -1)
    w = sel / (sel.sum(-1, keepdims=True) + 1e-8)            # [NTOK, 2]
    return idx, w.astype(np.float32)


def kernel(x, gate_w, gate_bank, up_bank, down_bank, _trace=False):
    _install_axon_ntff_hook()
    x = np.asarray(x, dtype=np.float32)
    gate_w = np.asarray(gate_w, dtype=np.float32)
    x2d = np.ascontiguousarray(x.reshape(NTOK, D))

    idx, w = _routing(x2d, gate_w)

    # Token lists per expert.
    tok_idx = []
    tok_w = []
    for e in range(E):
        hit = (idx == e)                        # [NTOK, 2]
        rows = np.nonzero(hit.any(-1))[0]
        tok_idx.append(rows)
        tok_w.append(w[rows, np.argmax(hit[rows], axis=-1)])
    nmax = max(len(r) for r in tok_idx)
    npad = ((nmax + 127) // 128) * 128

    key = npad
    if key not in _NC_CACHE:
        _NC_CACHE[key] = build_nc(npad)
    nc = _NC_CACHE[key]

    in_maps = []
    for e in range(E):
        rows = tok_idx[e]
        xg = np.zeros((npad, D), np.float32)
        xg[: len(rows)] = x2d[rows]
        wr = np.zeros((1, npad), np.float32)
        wr[0, : len(rows)] = tok_w[e]
        in_maps.append({
            "xgT": np.ascontiguousarray(xg.T),
            "gb": np.ascontiguousarray(gate_bank[e], dtype=np.float32),
            "ub": np.ascontiguousarray(up_bank[e], dtype=np.float32),
            "db": np.ascontiguousarray(down_bank[e], dtype=np.float32),
            "wrow": wr,
            "onesd": np.ones((1, 128), np.float32),
        })

    res = bass_utils.run_bass_kernel_spmd(
        nc, in_maps, core_ids=list(range(8)), trace=_trace)

    y = np.zeros((NTOK, D), np.float32)
    for e in range(E):
        rows = tok_idx[e]
        y[rows] += res.results[e]["ygT"][:, : len(rows)].T
    y = y.reshape(B, T, D)
    if _trace:
        return y, res
    return y


# revision 18
# speedup vs baseline: 1.1348x; 1.0422x over previous
"""MoE (B=2,T=2048,D=768,E=8,K=2,H=1536) Trainium2 kernel.

Sparse expert-parallel over the 8 NeuronCores: the host computes the gate
(softmax + top-2) in numpy, gathers the tokens routed to each expert, and
core e runs expert e's FFN only on its ~B*T*K/E gathered tokens. The
per-token gate weight is applied on device; the host scatter-adds the two
weighted expert outputs per token.

Activations stay feature-major (x^T [D, tok]) so gate/up banks [D,H] and
the down bank [H,D] are already in the stationary-operand (lhsT) layout the
PE wants — no transposes on device. The big GEMMs run in float32r (the PE's
single-pass fp32 mode, ~3.4x the 4-pass fp32 rate; per-GEMM rel err ~1.5e-4).
"""

import numpy as np

import concourse.bass as bass
import concourse.mybir as mybir
import concourse.tile as tile
from concourse import bass_utils

# Problem shape (hardcoded per contract).
B, T, D, E, H, KTOP = 2, 2048, 768, 8, 1536, 2
NTOK = B * T            # 4096 tokens
TOK = 512               # max tokens per block
DC = D // 128           # 6 chunks of the D (contraction) dim
HC = H // 128           # 12 chunks of the H dim
F32 = mybir.dt.float32
F32R = mybir.dt.float32r


def _install_axon_ntff_hook():
    """Best-effort: register the antenv.axon_hooks NTFF profile hook that the
    agent image lacks, so trace=True (or BASS_TRACE=1) can profile under axon.
    Never raises."""
    try:
        import sys, types, contextlib, ctypes  # noqa: PLC0415
        import antenv  # noqa: PLC0415
        if "antenv.axon_hooks" in sys.modules:
            return
        _HOOK = [None]
        mod = types.ModuleType("antenv.axon_hooks")
        mod.set_axon_ntff_profile_hook = lambda h: _HOOK.__setitem__(0, h)
        mod.get_axon_ntff_profile_hook = lambda: _HOOK[0]
        sys.modules["antenv.axon_hooks"] = mod
        antenv.axon_hooks = mod

        lib = ctypes.CDLL("/opt/axon/libaxon_pjrt.so")
        if not hasattr(lib, "axon_start_nrt_profile"):
            return
        lib.axon_start_nrt_profile.argtypes = [
            ctypes.POINTER(ctypes.c_int64), ctypes.c_size_t]
        lib.axon_start_nrt_profile.restype = ctypes.c_int64
        lib.axon_stop_nrt_profile.argtypes = [ctypes.c_char_p]
        lib.axon_stop_nrt_profile.restype = ctypes.c_int64

        @contextlib.contextmanager
        def _hook(output_dir, device_ids):
            import jax  # noqa: PLC0415
            jax.devices()
            if device_ids:
                ids = (ctypes.c_int64 * len(device_ids))(*device_ids)
                rc = lib.axon_start_nrt_profile(ids, len(device_ids))
            else:
                rc = lib.axon_start_nrt_profile(None, 0)
            if rc != 0:
                raise RuntimeError(f"axon_start_nrt_profile rc={rc}")
            try:
                yield
            finally:
                lib.axon_stop_nrt_profile(str(output_dir).encode())

        mod.set_axon_ntff_profile_hook(_hook)
    except Exception:
        pass


def _split_multiwaits(nc):
    """This walrus build only supports one sync-wait per instruction; move
    extra waits onto preceding NOPs on the same engine."""
    for fn in nc.m.functions:
        for bb in fn.blocks:
            out = []
            for ins in bb.instructions:
                si = ins.sync_info
                if si is not None and si.on_wait is not None and len(si.on_wait) > 1:
                    waits = list(si.on_wait)
                    for i, w in enumerate(waits[:-1]):
                        out.append(mybir.InstNoOp(
                            name=f"{ins.name}-sw{i}",
                            engine=ins.engine,
                            sync_info=mybir.SyncInfo(on_wait=[w], on_update=[]),
                        ))
                    si.on_wait = [waits[-1]]
                    ins.sync_info = si
                out.append(ins)
            bb.instructions = out
    return nc


def build_nc(npad):
    """Expert FFN on `npad` gathered tokens (feature-major, f32r GEMMs)."""
    # Equal-ish blocks of at most TOK tokens (multiples of 128): balanced
    # blocks beat [512, 512, remainder] because per-block matmul count is
    # fixed while per-matmul cost scales with N.
    ntile = npad // 128
    nblk = -(-ntile // (TOK // 128))
    sizes = [(ntile // nblk + (1 if i < ntile % nblk else 0)) * 128
             for i in range(nblk)]
    blocks = []
    off = 0
    for s in sizes:
        blocks.append((off, s))
        off += s

    nc = bass.Bass()
    xgT = nc.dram_tensor("xgT", [D, npad], F32R, kind="ExternalInput")
    gb = nc.dram_tensor("gb", [D, H], F32R, kind="ExternalInput")
    ub = nc.dram_tensor("ub", [D, H], F32R, kind="ExternalInput")
    db = nc.dram_tensor("db", [H, D], F32R, kind="ExternalInput")
    wrow = nc.dram_tensor("wrow", [1, npad], F32R, kind="ExternalInput")
    onesd = nc.dram_tensor("onesd", [1, 128], F32R, kind="ExternalInput")
    ygT = nc.dram_tensor("ygT", [D, npad], F32, kind="ExternalOutput")

    xgT_r = xgT.rearrange("(c p) t -> p c t", p=128)   # [128, DC, npad]
    gb_r = gb.rearrange("(c p) h -> p c h", p=128)     # [128, DC, H]
    ub_r = ub.rearrange("(c p) h -> p c h", p=128)
    db_r = db.rearrange("(c p) d -> p c d", p=128)     # [128, HC, D]
    ygT_r = ygT.rearrange("(c p) t -> p c t", p=128)

    with tile.TileContext(nc) as tc:
        with (
            tc.tile_pool(name="wts", bufs=1) as wts,
            tc.tile_pool(name="xp", bufs=2) as xp,
            tc.tile_pool(name="hp", bufs=18) as hp,
            tc.tile_pool(name="sap", bufs=2) as sap,
            tc.tile_pool(name="yp", bufs=3) as yp,
            tc.tile_pool(name="wsp", bufs=2) as wsp,
            tc.tile_pool(name="ps", bufs=8, space="PSUM") as ps,
        ):
            # Resident expert banks (f32r straight from DRAM), chunked and on
            # the ACT HWDGE ring (nc.scalar) so the x-block / output DMAs on
            # the SP ring (nc.sync) don't queue behind 14 MB of weights.
            # gb/ub interleave per D-chunk so GEMM1's k-loop streams as the
            # chunks land; db follows (needed only when GEMM2 starts).
            ones_sb = wts.tile([1, 128], F32R)
            nc.gpsimd.dma_start(ones_sb[:], onesd[:])
            wrow_sb = wts.tile([1, npad], F32R)
            nc.gpsimd.dma_start(wrow_sb[:], wrow[:])
            gb_k = [wts.tile([128, H], F32R, tag=f"gb{k}", name=f"gb{k}")
                    for k in range(DC)]
            ub_k = [wts.tile([128, H], F32R, tag=f"ub{k}", name=f"ub{k}")
                    for k in range(DC)]
            for k in range(DC):
                nc.gpsimd.dma_start(gb_k[k][:], gb_r[:, k, :])
                nc.gpsimd.dma_start(ub_k[k][:], ub_r[:, k, :])
            db_k = [wts.tile([128, D], F32R, tag=f"db{k}", name=f"db{k}")
                    for k in range(HC)]
            for k in range(HC):
                nc.gpsimd.dma_start(db_k[k][:], db_r[:, k, :])

            for off, tb in blocks:
                blk = slice(off, off + tb)

                xb = xp.tile([128, DC, tb], F32R, tag="xb")
                nc.sync.dma_start(xb[:], xgT_r[:, :, blk])

                # Broadcast the per-token gate weight across 128 partitions:
                # W[p, t] = wrow[t] via ones[1,128].T @ wrow[1, tb].
                w_ps = ps.tile([128, tb], F32, tag="ps")
                nc.tensor.matmul(w_ps[:], ones_sb[:], wrow_sb[:, blk],
                                 start=True, stop=True)
                wsb = wsp.tile([128, tb], F32)
                nc.vector.tensor_copy(wsb[:], w_ps[:])

                # h = silu(x@gb) * (x@ub), feature-major [H, tb]
                hts = []
                for ht in range(HC):
                    hsl = slice(ht * 128, (ht + 1) * 128)
                    a_ps = ps.tile([128, tb], F32, tag="ps")
                    for k in range(DC):
                        nc.tensor.matmul(a_ps[:], gb_k[k][:, hsl],
                                         xb[:, k, :],
                                         start=(k == 0), stop=(k == DC - 1))
                    u_ps = ps.tile([128, tb], F32, tag="ps")
                    for k in range(DC):
                        nc.tensor.matmul(u_ps[:], ub_k[k][:, hsl],
                                         xb[:, k, :],
                                         start=(k == 0), stop=(k == DC - 1))
                    sa = sap.tile([128, tb], F32)
                    nc.scalar.activation(sa[:], a_ps[:],
                                         mybir.ActivationFunctionType.Silu)
                    hch = hp.tile([128, tb], F32R, tag="h")
                    nc.vector.tensor_mul(hch[:], sa[:],
                                         u_ps[:])
                    hts.append(hch)

                # y^T = db^T @ h, scaled by the gate weight
                for dt in range(DC):
                    dsl = slice(dt * 128, (dt + 1) * 128)
                    y_ps = ps.tile([128, tb], F32, tag="ps")
                    for hk in range(HC):
                        nc.tensor.matmul(y_ps[:], db_k[hk][:, dsl],
                                         hts[hk][:],
                                         start=(hk == 0), stop=(hk == HC - 1))
                    ysb = yp.tile([128, tb], F32)
                    nc.vector.tensor_mul(ysb[:], y_ps[:],
                                         wsb[:])
                    nc.sync.dma_start(ygT_r[:, dt, blk], ysb[:])

    return _split_multiwaits(nc)


_NC_CACHE = {}


def _routing(x2d, gate_w):
    """Replicates the reference gate: softmax over E, top-2, renormalize."""
    logits = x2d @ gate_w.T                                  # [NTOK, E] f32
    lmax = logits.max(-1, keepdims=True)
    p = np.exp(logits - lmax)
    p = p / p.sum(-1, keepdims=True)
    idx = np.argsort(-p, axis=-1, kind="stable")[:, :KTOP]   # [NTOK, 2]
    sel = np.take_along_axis(p, idx, -1)
    w = sel / (sel.sum(-1, keepdims=True) + 1e-8)            # [NTOK, 2]
    return idx, w.astype(np.float32)


def kernel(x, gate_w, gate_bank, up_bank, down_bank, _trace=False):
    _install_axon_ntff_hook()
    x = np.asarray(x, dtype=np.float32)
    gate_w = np.asarray(gate_w, dtype=np.float32)
    x2d = np.ascontiguousarray(x.reshape(NTOK, D))

    idx, w = _routing(x2d, gate_w)

    # Token lists per expert.
    tok_idx = []
    tok_w = []
    for e in range(E):
        hit = (idx == e)                        # [NTOK, 2]
        rows = np.nonzero(hit.any(-1))[0]
        tok_idx.append(rows)
        tok_w.append(w[rows, np.argmax(hit[rows], axis=-1)])
    nmax = max(len(r) for r in tok_idx)
    npad = ((nmax + 127) // 128) * 128

    key = npad
    if key not in _NC_CACHE:
        _NC_CACHE[key] = build_nc(npad)
    nc = _NC_CACHE[key]

    in_maps = []
    for e in range(E):
        rows = tok_idx[e]
        xg = np.zeros((npad, D), np.float32)
        xg[: len(rows)] = x2d[rows]
        wr = np.zeros((1, npad), np.float32)
        wr[0, : len(rows)] = tok_w[e]
        in_maps.append({
            "xgT": np.ascontiguousarray(xg.T),
            "gb": np.ascontiguousarray(gate_bank[e], dtype=np.float32),
            "ub": np.ascontiguousarray(up_bank[e], dtype=np.float32),
            "db": np.ascontiguousarray(down_bank[e], dtype=np.float32),
            "wrow": wr,
            "onesd": np.ones((1, 128), np.float32),
        })

    res = bass_utils.run_bass_kernel_spmd(
        nc, in_maps, core_ids=list(range(8)), trace=_trace)

    y = np.zeros((NTOK, D), np.float32)
    for e in range(E):
        rows = tok_idx[e]
        y[rows] += res.results[e]["ygT"][:, : len(rows)].T
    y = y.reshape(B, T, D)
    if _trace:
        return y, res
    return y


# revision 20
# speedup vs baseline: 1.2033x; 1.0603x over previous
"""MoE (B=2,T=2048,D=768,E=8,K=2,H=1536) Trainium2 kernel.

Sparse expert-parallel over the 8 NeuronCores: the host computes the gate
(softmax + top-2) in numpy, gathers the tokens routed to each expert, and
core e runs expert e's FFN only on its ~B*T*K/E gathered tokens. The
per-token gate weight is applied on device; the host scatter-adds the two
weighted expert outputs per token.

Activations stay feature-major (x^T [D, tok]) so gate/up banks [D,H] and
the down bank [H,D] are already in the stationary-operand (lhsT) layout the
PE wants — no transposes on device. The big GEMMs run in float32r (the PE's
single-pass fp32 mode, ~3.4x the 4-pass fp32 rate; per-GEMM rel err ~1.5e-4).
"""

import numpy as np

import concourse.bass as bass
import concourse.mybir as mybir
import concourse.tile as tile
from concourse import bass_utils

# Problem shape (hardcoded per contract).
B, T, D, E, H, KTOP = 2, 2048, 768, 8, 1536, 2
NTOK = B * T            # 4096 tokens
TOK = 512               # max tokens per block
DC = D // 128           # 6 chunks of the D (contraction) dim
HC = H // 128           # 12 chunks of the H dim
F32 = mybir.dt.float32
F32R = mybir.dt.float32r


def _install_axon_ntff_hook():
    """Best-effort: register the antenv.axon_hooks NTFF profile hook that the
    agent image lacks, so trace=True (or BASS_TRACE=1) can profile under axon.
    Never raises."""
    try:
        import sys, types, contextlib, ctypes  # noqa: PLC0415
        import antenv  # noqa: PLC0415
        if "antenv.axon_hooks" in sys.modules:
            return
        _HOOK = [None]
        mod = types.ModuleType("antenv.axon_hooks")
        mod.set_axon_ntff_profile_hook = lambda h: _HOOK.__setitem__(0, h)
        mod.get_axon_ntff_profile_hook = lambda: _HOOK[0]
        sys.modules["antenv.axon_hooks"] = mod
        antenv.axon_hooks = mod

        lib = ctypes.CDLL("/opt/axon/libaxon_pjrt.so")
        if not hasattr(lib, "axon_start_nrt_profile"):
            return
        lib.axon_start_nrt_profile.argtypes = [
            ctypes.POINTER(ctypes.c_int64), ctypes.c_size_t]
        lib.axon_start_nrt_profile.restype = ctypes.c_int64
        lib.axon_stop_nrt_profile.argtypes = [ctypes.c_char_p]
        lib.axon_stop_nrt_profile.restype = ctypes.c_int64

        @contextlib.contextmanager
        def _hook(output_dir, device_ids):
            import jax  # noqa: PLC0415
            jax.devices()
            if device_ids:
                ids = (ctypes.c_int64 * len(device_ids))(*device_ids)
                rc = lib.axon_start_nrt_profile(ids, len(device_ids))
            else:
                rc = lib.axon_start_nrt_profile(None, 0)
            if rc != 0:
                raise RuntimeError(f"axon_start_nrt_profile rc={rc}")
            try:
                yield
            finally:
                lib.axon_stop_nrt_profile(str(output_dir).encode())

        mod.set_axon_ntff_profile_hook(_hook)
    except Exception:
        pass


def _split_multiwaits(nc):
    """This walrus build only supports one sync-wait per instruction; move
    extra waits onto preceding NOPs on the same engine."""
    for fn in nc.m.functions:
        for bb in fn.blocks:
            out = []
            for ins in bb.instructions:
                si = ins.sync_info
                if si is not None and si.on_wait is not None and len(si.on_wait) > 1:
                    waits = list(si.on_wait)
                    for i, w in enumerate(waits[:-1]):
                        out.append(mybir.InstNoOp(
                            name=f"{ins.name}-sw{i}",
                            engine=ins.engine,
                            sync_info=mybir.SyncInfo(on_wait=[w], on_update=[]),
                        ))
                    si.on_wait = [waits[-1]]
                    ins.sync_info = si
                out.append(ins)
            bb.instructions = out
    return nc


def build_nc(npad):
    """Expert FFN on `npad` gathered tokens (feature-major, f32r GEMMs)."""
    # Equal-ish blocks of at most TOK tokens (multiples of 128): balanced
    # blocks beat [512, 512, remainder] because per-block matmul count is
    # fixed while per-matmul cost scales with N.
    ntile = npad // 128
    nblk = -(-ntile // (TOK // 128))
    sizes = [(ntile // nblk + (1 if i < ntile % nblk else 0)) * 128
             for i in range(nblk)]
    blocks = []
    off = 0
    for s in sizes:
        blocks.append((off, s))
        off += s

    nc = bass.Bass()
    xgT = nc.dram_tensor("xgT", [D, npad], F32R, kind="ExternalInput")
    gb = nc.dram_tensor("gb", [D, H], F32R, kind="ExternalInput")
    ub = nc.dram_tensor("ub", [D, H], F32R, kind="ExternalInput")
    db = nc.dram_tensor("db", [H, D], F32R, kind="ExternalInput")
    wrow = nc.dram_tensor("wrow", [1, npad], F32R, kind="ExternalInput")
    onesd = nc.dram_tensor("onesd", [1, 128], F32R, kind="ExternalInput")
    ygT = nc.dram_tensor("ygT", [D, npad], F32, kind="ExternalOutput")

    xgT_r = xgT.rearrange("(c p) t -> p c t", p=128)   # [128, DC, npad]
    gb_r = gb.rearrange("(c p) h -> p c h", p=128)     # [128, DC, H]
    ub_r = ub.rearrange("(c p) h -> p c h", p=128)
    db_r = db.rearrange("(c p) d -> p c d", p=128)     # [128, HC, D]
    ygT_r = ygT.rearrange("(c p) t -> p c t", p=128)

    with tile.TileContext(nc) as tc:
        with (
            tc.tile_pool(name="wts", bufs=1) as wts,
            tc.tile_pool(name="xp", bufs=2) as xp,
            tc.tile_pool(name="hp", bufs=18) as hp,
            tc.tile_pool(name="sap", bufs=2) as sap,
            tc.tile_pool(name="yp", bufs=3) as yp,
            tc.tile_pool(name="wsp", bufs=2) as wsp,
            tc.tile_pool(name="ps", bufs=8, space="PSUM") as ps,
        ):
            # Resident expert banks (f32r straight from DRAM) via the SWDGE
            # (gpsimd) queue so the SP HWDGE ring (x-blocks / outputs) stays
            # clean and the ACT engine queue never stalls on DMA dispatches
            # (which would delay every SILU behind them). Banks are split
            # into H-halves: GEMM1 runs at full rate on half a bank while
            # the rest streams in.
            HHALF = H // 2
            gb_h = [wts.tile([128, DC, HHALF], F32R, tag=f"gbh{i}",
                             name=f"gbh{i}") for i in range(2)]
            ub_h = [wts.tile([128, DC, HHALF], F32R, tag=f"ubh{i}",
                             name=f"ubh{i}") for i in range(2)]
            db_h = [wts.tile([128, HC // 2, D], F32R, tag=f"dbh{i}",
                             name=f"dbh{i}") for i in range(2)]
            ones_sb = wts.tile([1, 128], F32R)
            wrow_sb = wts.tile([1, npad], F32R)
            nc.gpsimd.dma_start(gb_h[0][:], gb_r[:, :, 0:HHALF])
            nc.gpsimd.dma_start(ub_h[0][:], ub_r[:, :, 0:HHALF])
            nc.gpsimd.dma_start(ones_sb[:], onesd[:])
            nc.gpsimd.dma_start(wrow_sb[:], wrow[:])
            nc.gpsimd.dma_start(gb_h[1][:], gb_r[:, :, HHALF:H])
            nc.gpsimd.dma_start(ub_h[1][:], ub_r[:, :, HHALF:H])
            nc.gpsimd.dma_start(db_h[0][:], db_r[:, 0:HC // 2, :])
            nc.gpsimd.dma_start(db_h[1][:], db_r[:, HC // 2:HC, :])

            for off, tb in blocks:
                blk = slice(off, off + tb)

                xb = xp.tile([128, DC, tb], F32R, tag="xb")
                nc.sync.dma_start(xb[:], xgT_r[:, :, blk])

                # h = silu(x@gb) * (x@ub), feature-major [H, tb]
                hts = []
                for ht in range(HC):
                    half, hh = divmod(ht, HC // 2)
                    hsl = slice(hh * 128, (hh + 1) * 128)
                    a_ps = ps.tile([128, tb], F32, tag="ps")
                    for k in range(DC):
                        nc.tensor.matmul(a_ps[:], gb_h[half][:, k, hsl],
                                         xb[:, k, :],
                                         start=(k == 0), stop=(k == DC - 1))
                    u_ps = ps.tile([128, tb], F32, tag="ps")
                    for k in range(DC):
                        nc.tensor.matmul(u_ps[:], ub_h[half][:, k, hsl],
                                         xb[:, k, :],
                                         start=(k == 0), stop=(k == DC - 1))
                    sa = sap.tile([128, tb], F32)
                    nc.scalar.activation(sa[:], a_ps[:],
                                         mybir.ActivationFunctionType.Silu)
                    hch = hp.tile([128, tb], F32R, tag="h")
                    nc.vector.tensor_mul(hch[:], sa[:],
                                         u_ps[:])
                    hts.append(hch)

                # Broadcast the per-token gate weight across 128 partitions:
                # W[p, t] = wrow[t] via ones[1,128].T @ wrow[1, tb]. Emitted
                # after GEMM1 so the in-order PE never waits on it at start.
                w_ps = ps.tile([128, tb], F32, tag="ps")
                nc.tensor.matmul(w_ps[:], ones_sb[:], wrow_sb[:, blk],
                                 start=True, stop=True)
                wsb = wsp.tile([128, tb], F32)
                nc.vector.tensor_copy(wsb[:], w_ps[:])

                # y^T = db^T @ h, scaled by the gate weight
                for dt in range(DC):
                    dsl = slice(dt * 128, (dt + 1) * 128)
                    y_ps = ps.tile([128, tb], F32, tag="ps")
                    for hk in range(HC):
                        half, kk = divmod(hk, HC // 2)
                        nc.tensor.matmul(y_ps[:], db_h[half][:, kk, dsl],
                                         hts[hk][:],
                                         start=(hk == 0), stop=(hk == HC - 1))
                    ysb = yp.tile([128, tb], F32)
                    nc.vector.tensor_mul(ysb[:], y_ps[:],
                                         wsb[:])
                    nc.sync.dma_start(ygT_r[:, dt, blk], ysb[:])

    return _split_multiwaits(nc)


_NC_CACHE = {}


def _routing(x2d, gate_w):
    """Replicates the reference gate: softmax over E, top-2, renormalize."""
    logits = x2d @ gate_w.T                                  # [NTOK, E] f32
    lmax = logits.max(-1, keepdims=True)
    p = np.exp(logits - lmax)
    p = p / p.sum(-1, keepdims=True)
    idx = np.argsort(-p, axis=-1, kind="stable")[:, :KTOP]   # [NTOK, 2]
    sel = np.take_along_axis(p, idx, -1)
    w = sel / (sel.sum(-1, keepdims=True) + 1e-8)            # [NTOK, 2]
    return idx, w.astype(np.float32)


def kernel(x, gate_w, gate_bank, up_bank, down_bank, _trace=False):
    _install_axon_ntff_hook()
    x = np.asarray(x, dtype=np.float32)
    gate_w = np.asarray(gate_w, dtype=np.float32)
    x2d = np.ascontiguousarray(x.reshape(NTOK, D))

    idx, w = _routing(x2d, gate_w)

    # Token lists per expert.
    tok_idx = []
    tok_w = []
    for e in range(E):
        hit = (idx == e)                        # [NTOK, 2]
        rows = np.nonzero(hit.any(-1))[0]
        tok_idx.append(rows)
        tok_w.append(w[rows, np.argmax(hit[rows], axis=-1)])
    nmax = max(len(r) for r in tok_idx)
    npad = ((nmax + 127) // 128) * 128

    key = npad
    if key not in _NC_CACHE:
        _NC_CACHE[key] = build_nc(npad)
    nc = _NC_CACHE[key]

    in_maps = []
    for e in range(E):
        rows = tok_idx[e]
        xg = np.zeros((npad, D), np.float32)
        xg[: len(rows)] = x2d[rows]
        wr = np.zeros((1, npad), np.float32)
        wr[0, : len(rows)] = tok_w[e]
        in_maps.append({
            "xgT": np.ascontiguousarray(xg.T),
            "gb": np.ascontiguousarray(gate_bank[e], dtype=np.float32),
            "ub": np.ascontiguousarray(up_bank[e], dtype=np.float32),
            "db": np.ascontiguousarray(down_bank[e], dtype=np.float32),
            "wrow": wr,
            "onesd": np.ones((1, 128), np.float32),
        })

    res = bass_utils.run_bass_kernel_spmd(
        nc, in_maps, core_ids=list(range(8)), trace=_trace)

    y = np.zeros((NTOK, D), np.float32)
    for e in range(E):
        rows = tok_idx[e]
        y[rows] += res.results[e]["ygT"][:, : len(rows)].T
    y = y.reshape(B, T, D)
    if _trace:
        return y, res
    return y
